# revision 16
# baseline (speedup 1.0000x reference)
"""GCN message-passing layer (GCNConv + skip + BatchNorm + ReLU) on 8 TRN2 cores.

Strategy v3 (CSR segment-sum, transposed output, no on-device gather):
  - Nodes sharded across 8 cores (12500 each, padded to 12544 = 98*128),
    placed degree-descending into 128-row tiles so each tile's max
    in-degree K_t is near its mean (CSR padding ~10%).
  - Host prep (integer indexing / byte layout only): edges+self-loops CSR
    grouped by target; per TILE-PAIR the source rows of x (bf16) are laid
    out dense [128 slots, 2, 64 feats, Kp] (j contiguous), shipped with
    per-edge source degrees (1e30 padding => rsqrt ~ 0 kills pads).
  - Device per pair: one DMA; one tensor_tensor mult by dinv_src
    (broadcast over feats; alternates GPSIMD/DVE); ONE 2x-mode bf16
    tensor_reduce over K = the segment sum for both tiles. Per tile: PE
    matmul agg^T @ diag(dinv_tgt) (transpose + target normalization in
    one), evac into stacked [aggT ; xT]; PE matmul lhsT=[W;skipW] gives
    v^T = (agg@W + x@skipW)^T with BN feature dim on partitions; ACT evac
    with free accum_out = BN sum. Sum-of-squares, BN AllReduce, and the
    affine+ReLU apply run as a handful of wide whole-buffer ops.
  - All float arithmetic runs on device; the host only reorders input
    bytes (bf16 cast) and computes integer degrees (+ 1/sqrt(deg) diag,
    same class as the baseline's host-built float index tables).
"""

import numpy as np
import ml_dtypes

P = 128
_BF16 = ml_dtypes.bfloat16

_KCACHE = {}


def _host_prep(x, edge_index, W, skip_W, gamma, beta, M, IN, OUT):
    N = x.shape[0]
    SH = N // M
    T = -(-SH // P)
    SHP = T * P
    assert T % 2 == 0
    NP = T // 2

    row = edge_index[0].astype(np.int64)
    col = edge_index[1].astype(np.int64)
    loops = np.arange(N, dtype=np.int64)
    row_f = np.concatenate([row, loops])
    col_f = np.concatenate([col, loops])

    deg_i = np.bincount(col_f, minlength=N)          # >=1 (self loops)
    deg_f = deg_i.astype(np.float32)

    # degree-descending node placement per core: rank r -> (tile r//P, slot r%P)
    node_pos = np.empty(N, dtype=np.int64)
    orders = []
    Kt_cores = []
    for m in range(M):
        dg = deg_i[m * SH:(m + 1) * SH]
        order = np.argsort(-dg, kind="stable")
        ranks = np.empty(SH, dtype=np.int64)
        ranks[order] = np.arange(SH)
        node_pos[m * SH:(m + 1) * SH] = ranks
        orders.append(order)
        dgs = np.zeros(SHP, dtype=np.int64)
        dgs[:SH] = dg[order]
        Kt_cores.append(dgs.reshape(T, P).max(axis=1))
    Kt = np.maximum.reduce(Kt_cores)
    Kp = np.maximum(Kt.reshape(NP, 2).max(axis=1), 4)
    Kp = ((Kp + 3) // 4) * 4                         # per-PAIR K, mult of 4
    opf = np.zeros(NP + 1, dtype=np.int64)
    np.cumsum(Kp, out=opf[1:])
    SKP = int(opf[-1])                               # sum of pair Ks

    # CSR by target node
    eorder = np.argsort(col_f, kind="stable")
    row_s = row_f[eorder]
    starts = np.zeros(N + 1, dtype=np.int64)
    np.cumsum(deg_i, out=starts[1:])

    x_bf = x.astype(_BF16)
    WS = np.concatenate([np.asarray(W), np.asarray(skip_W)], axis=0).astype(_BF16)
    dinv_all = (1.0 / np.sqrt(deg_f)).astype(np.float32)

    in_maps = []
    for m in range(M):
        order = orders[m]
        xgt = np.zeros((P, 2 * IN * SKP), dtype=_BF16)
        dege = np.full((P, 2 * SKP), 1e30, dtype=np.float32)
        diag = np.zeros((P, T * P), dtype=_BF16)
        ii = np.arange(P)
        for t in range(T):
            K = int(Kp[t // 2])
            o2 = int(2 * IN * opf[t // 2] + (t % 2) * IN * K)
            od = int(2 * opf[t // 2] + (t % 2) * K)
            rr = np.arange(t * P, (t + 1) * P)
            vslot = rr < SH
            ln = np.where(vslot, order[np.minimum(rr, SH - 1)], 0)
            gn = m * SH + ln
            cnt = np.where(vslot, deg_i[gn], 0)
            st = starts[gn]
            j = np.arange(K, dtype=np.int64)[None, :]
            vm = j < cnt[:, None]                    # [P, K]
            eidx = st[:, None] + np.minimum(j, np.maximum(cnt[:, None] - 1, 0))
            srcs = np.where(vm, row_s[eidx], 0)
            xg = x_bf[srcs]                          # [P, K, IN]
            xgt[:, o2:o2 + IN * K] = xg.transpose(0, 2, 1).reshape(P, IN * K)
            dege[:, od:od + K] = np.where(vm, deg_f[srcs], 1e30)
            dv_t = np.where(vslot, dinv_all[gn], 1.0).astype(_BF16)
            diag[ii, t * P + ii] = dv_t

        # stacked transposed x: partitions IN..2*IN hold x^T at permuted slots
        xperm = np.zeros((SHP, IN), dtype=_BF16)
        xperm[node_pos[m * SH:(m + 1) * SH]] = x_bf[m * SH:(m + 1) * SH]
        xstack = np.zeros((P, T * P), dtype=_BF16)
        xstack[IN:2 * IN, :] = xperm.T

        in_maps.append({
            "xgt": np.ascontiguousarray(xgt),
            "dege": np.ascontiguousarray(dege),
            "xstack": np.ascontiguousarray(xstack),
            "diag": np.ascontiguousarray(diag),
            "WS": np.ascontiguousarray(WS),
            "gammac": np.ascontiguousarray(np.asarray(gamma, np.float32).reshape(1, OUT)),
            "betac": np.ascontiguousarray(np.asarray(beta, np.float32).reshape(1, OUT)),
        })
    return in_maps, tuple(int(k) for k in Kp), node_pos, SH, T, SHP


def _build(M, N, IN, OUT, T, Kp, debug_stop="full"):
    from concourse import bacc, mybir, tile

    dt = mybir.dt
    Alu = mybir.AluOpType
    Act = mybir.ActivationFunctionType

    SHP = T * P
    NP = T // 2
    BN_EPS = 1e-5
    opf = np.zeros(NP + 1, dtype=np.int64)
    np.cumsum(np.asarray(Kp), out=opf[1:])
    SKP = int(opf[-1])

    nc = bacc.Bacc("TRN2", target_bir_lowering=False, debug=False,
                   num_devices=M)

    xgt_d = nc.dram_tensor("xgt", [P, 2 * IN * SKP], dt.bfloat16,
                           kind="ExternalInput")
    dege_d = nc.dram_tensor("dege", [P, 2 * SKP], dt.float32,
                            kind="ExternalInput")
    xstack_d = nc.dram_tensor("xstack", [P, T * P], dt.bfloat16,
                              kind="ExternalInput")
    diag_d = nc.dram_tensor("diag", [P, T * P], dt.bfloat16,
                            kind="ExternalInput")
    WS_d = nc.dram_tensor("WS", [2 * IN, OUT], dt.bfloat16, kind="ExternalInput")
    gammac_d = nc.dram_tensor("gammac", [1, OUT], dt.float32,
                              kind="ExternalInput")
    betac_d = nc.dram_tensor("betac", [1, OUT], dt.float32,
                             kind="ExternalInput")
    out_d = nc.dram_tensor("out", [P, T * P], dt.bfloat16,
                           kind="ExternalOutput")

    st_local = nc.dram_tensor("st_local", [2, OUT], dt.float32)
    st_global = nc.dram_tensor("st_global", [2, OUT], dt.float32,
                               addr_space="Shared")
    rg = [list(range(M))]

    NCH = 7                       # sum-of-squares chunking
    assert T % NCH == 0
    CH = T // NCH

    with tile.TileContext(nc) as tc:
        with (
            tc.tile_pool(name="const", bufs=1) as cpool,
            tc.tile_pool(name="gload", bufs=3) as gpool,
            tc.tile_pool(name="gmul", bufs=2) as mpool,
            tc.tile_pool(name="hfold", bufs=2) as hpool,
            tc.tile_pool(name="aggp", bufs=3) as apool,
            tc.tile_pool(name="sqp", bufs=2) as qpool,
            tc.tile_pool(name="ps_tr", bufs=2, space="PSUM") as ps_tr,
            tc.tile_pool(name="ps_out", bufs=3, space="PSUM") as ps_out,
        ):
            # ---- constants / persistent ----
            xstack_sb = cpool.tile([P, T * P], dt.bfloat16, tag="xstack")
            nc.sync.dma_start(xstack_sb[:], xstack_d[:, :])
            diag_sb = cpool.tile([P, T * P], dt.bfloat16, tag="diag")
            nc.sync.dma_start(diag_sb[:], diag_d[:, :])
            WS_sb = cpool.tile([2 * IN, OUT], dt.bfloat16, tag="WS")
            nc.sync.dma_start(WS_sb[:], WS_d[:, :])
            dege_sb = cpool.tile([P, 2 * SKP], dt.float32, tag="dege")
            nc.sync.dma_start(dege_sb[:], dege_d[:, :])
            gammar_sb = cpool.tile([1, OUT], dt.float32, tag="gammar")
            nc.sync.dma_start(gammar_sb[:], gammac_d[:, :])
            betar_sb = cpool.tile([1, OUT], dt.float32, tag="betar")
            nc.sync.dma_start(betar_sb[:], betac_d[:, :])
            from concourse.masks import make_identity
            identf = cpool.tile([P, P], dt.float32, tag="identf")
            make_identity(nc, identf[:])
            ones11 = cpool.tile([1, 1], dt.float32, tag="ones11")
            nc.vector.memset(ones11[:], 1.0)

            vbuf = cpool.tile([P, T * P], dt.bfloat16, tag="vbuf")
            accs = cpool.tile([P, T], dt.float32, tag="accs")
            accq = cpool.tile([P, NCH], dt.float32, tag="accq")

            # dinv_src = sqrt(1/deg) (ACT Rsqrt banned for accuracy)
            rec_e = cpool.tile([P, 2 * SKP], dt.float32, tag="rec_e")
            dinve = cpool.tile([P, 2 * SKP], dt.bfloat16, tag="dinve")
            dq = 2 * SKP // 4
            for qi in range(4):
                lo = qi * dq
                hi = 2 * SKP if qi == 3 else (qi + 1) * dq
                nc.vector.reciprocal(rec_e[:, lo:hi], dege_sb[:, lo:hi])
                nc.scalar.activation(dinve[:, lo:hi], rec_e[:, lo:hi],
                                     Act.Sqrt)

            # ---- main loop over tile pairs (software-pipelined by 1) ----
            hq = [None] * NP

            def stage_front(p):
                K = int(Kp[p])
                o = int(opf[p])
                g = gpool.tile([P, 2, IN, K], dt.bfloat16, tag="g")
                nc.sync.dma_start(g[:], xgt_d[:, 2 * IN * o:2 * IN * (o + K)])
                gm = mpool.tile([P, 2, IN, K], dt.bfloat16, tag="gm")
                dv = (dinve[:, 2 * o:2 * (o + K)]
                      .rearrange("p (two k) -> p two k", two=2)
                      .unsqueeze(2).broadcast_to([P, 2, IN, K]))
                nc.vector.tensor_tensor(gm[:], g[:], dv, Alu.mult)
                K2 = K // 2
                h = hpool.tile([P, 2, IN, K2], dt.bfloat16, tag="h")
                nc.gpsimd.tensor_tensor(h[:], gm[:, :, :, 0:K2],
                                        gm[:, :, :, K2:K], Alu.add)
                hq[p] = h

            def stage_back(p):
                h = hq[p]
                hq[p] = None
                aggp = apool.tile([P, 2 * IN], dt.bfloat16, tag="aggp")
                with nc.allow_low_precision("bf16 agg; 2x-mode reduce"):
                    nc.vector.tensor_reduce(aggp[:], h[:],
                                            mybir.AxisListType.X, Alu.add)
                for half in (0, 1):
                    ti = 2 * p + half
                    pt = ps_tr.tile([IN, P], dt.float32, tag="pt")
                    nc.tensor.matmul(pt[:],
                                     lhsT=aggp[:, half * IN:(half + 1) * IN],
                                     rhs=diag_sb[:, ti * P:(ti + 1) * P],
                                     start=True, stop=True)
                    nc.scalar.copy(xstack_sb[0:IN, ti * P:(ti + 1) * P], pt[:])
                    po = ps_out.tile([P, P], dt.float32, tag="po")
                    nc.tensor.matmul(po[:], lhsT=WS_sb[:],
                                     rhs=xstack_sb[:, ti * P:(ti + 1) * P],
                                     start=True, stop=True)
                    nc.scalar.activation(vbuf[:, ti * P:(ti + 1) * P], po[:],
                                         Act.Copy,
                                         accum_out=accs[:, ti:ti + 1])
                if debug_stop == "full" and (2 * p + 2) % CH == 0:
                    c = (2 * p + 2) // CH - 1
                    scr = qpool.tile([P, CH * P], dt.bfloat16, tag="scr")
                    vsl = vbuf[:, c * CH * P:(c + 1) * CH * P]
                    nc.scalar.activation(scr[:], vsl, Act.Square,
                                         accum_out=accq[:, c:c + 1])

            stage_front(0)
            for p in range(1, NP):
                stage_front(p)
                stage_back(p - 1)
            stage_back(NP - 1)

            if debug_stop == "v":
                nc.sync.dma_start(out_d[:, :], vbuf[:])

            if debug_stop == "full":
                # ---- BN stats: sum of squares (chunked), totals, allreduce
                st2 = cpool.tile([P, 2], dt.float32, tag="st2")
                nc.vector.tensor_reduce(st2[:, 0:1], accs[:],
                                        mybir.AxisListType.X, Alu.add)
                nc.vector.tensor_reduce(st2[:, 1:2], accq[:],
                                        mybir.AxisListType.X, Alu.add)
                # -> row layout [2, P] so the collective sees a flat buffer
                pst = ps_tr.tile([2, P], dt.float32, tag="pst", bufs=1)
                nc.tensor.matmul(pst[:], lhsT=st2[:], rhs=identf[:],
                                 start=True, stop=True)
                st_sb = cpool.tile([2, P], dt.float32, tag="st_sb")
                nc.scalar.copy(st_sb[:], pst[:])
                nc.sync.dma_start(st_local[:, :], st_sb[:])
                nc.gpsimd.collective_compute(
                    "AllReduce", Alu.add, replica_groups=rg,
                    ins=[st_local.ap().opt()], outs=[st_global.ap().opt()])
                sg_sum = cpool.tile([1, OUT], dt.float32, tag="sg_sum")
                nc.sync.dma_start(sg_sum[:], st_global[0:1, :])
                sg_sq = cpool.tile([1, OUT], dt.float32, tag="sg_sq")
                nc.sync.dma_start(sg_sq[:], st_global[1:2, :])

                # BN coefficient math in row form [1, OUT]
                inv_n = 1.0 / float(N)
                mean_r = cpool.tile([1, OUT], dt.float32, tag="mean_r")
                nc.vector.tensor_scalar(mean_r[:], sg_sum[:], inv_n,
                                        None, Alu.mult)
                var_r = cpool.tile([1, OUT], dt.float32, tag="var_r")
                nc.vector.tensor_scalar(var_r[:], sg_sq[:], inv_n,
                                        None, Alu.mult)
                msq = cpool.tile([1, OUT], dt.float32, tag="msq")
                nc.vector.tensor_tensor(msq[:], mean_r[:], mean_r[:], Alu.mult)
                nc.vector.tensor_tensor(var_r[:], var_r[:], msq[:],
                                        Alu.subtract)
                nc.vector.tensor_scalar(var_r[:], var_r[:], BN_EPS, None,
                                        Alu.add)
                rvar = cpool.tile([1, OUT], dt.float32, tag="rvar")
                nc.vector.reciprocal(rvar[:], var_r[:])
                rstd = cpool.tile([1, OUT], dt.float32, tag="rstd")
                nc.scalar.activation(rstd[:], rvar[:], Act.Sqrt)
                a_row = cpool.tile([1, OUT], dt.float32, tag="a_row")
                nc.vector.tensor_tensor(a_row[:], gammar_sb[:], rstd[:],
                                        Alu.mult)
                ma = cpool.tile([1, OUT], dt.float32, tag="ma")
                nc.vector.tensor_tensor(ma[:], mean_r[:], a_row[:], Alu.mult)
                b_row = cpool.tile([1, OUT], dt.float32, tag="b_row")
                nc.vector.tensor_tensor(b_row[:], betar_sb[:], ma[:],
                                        Alu.subtract)
                # rows -> per-partition columns via 1-contraction matmuls
                pa = ps_out.tile([P, 1], dt.float32, tag="pa", bufs=1)
                nc.tensor.matmul(pa[:], lhsT=a_row[:], rhs=ones11[:],
                                 start=True, stop=True)
                a_col = cpool.tile([P, 1], dt.float32, tag="a_col")
                nc.scalar.copy(a_col[:], pa[:])
                pb = ps_tr.tile([P, 1], dt.float32, tag="pb", bufs=1)
                nc.tensor.matmul(pb[:], lhsT=b_row[:], rhs=ones11[:],
                                 start=True, stop=True)
                b_col = cpool.tile([P, 1], dt.float32, tag="b_col")
                nc.scalar.copy(b_col[:], pb[:])

                # ---- BN apply + ReLU in-place, batched output DMA ----
                nc.vector.tensor_scalar(vbuf[:], vbuf[:], a_col[:], b_col[:],
                                        Alu.mult, Alu.add)
                nc.vector.tensor_scalar(vbuf[:], vbuf[:], 0.0, None, Alu.max)
                H = T * P // 2
                nc.sync.dma_start(out_d[:, 0:H], vbuf[:, 0:H])
                nc.sync.dma_start(out_d[:, H:2 * H], vbuf[:, H:2 * H])

    nc.compile()
    return nc


def _run(nc, in_maps, M, trace=False):
    from concourse import bass_utils
    res = bass_utils.run_bass_kernel_spmd(
        nc, in_maps, core_ids=list(range(M)), trace=trace)
    return res


def kernel(x, edge_index, W, bias, skip_W, gamma, beta, _trace=False,
           _return_results=False, _debug_stop="full"):
    x = np.asarray(x, dtype=np.float32)
    edge_index = np.asarray(edge_index, dtype=np.int32)
    M = 8
    N, IN = x.shape
    OUT = np.asarray(W).shape[1]

    in_maps, Kp, node_pos, SH, T, SHP = _host_prep(
        x, edge_index, W, skip_W, gamma, beta, M, IN, OUT)
    key = (M, N, IN, OUT, T, Kp, _debug_stop)
    if key not in _KCACHE:
        _KCACHE[key] = _build(M, N, IN, OUT, T, Kp, debug_stop=_debug_stop)
    nc = _KCACHE[key]

    res = _run(nc, in_maps, M, trace=_trace)
    outs = []
    for m in range(M):
        arr = res.results[m]["out"].astype(np.float32)      # [P, T*P] v^T
        full_m = arr.reshape(OUT, T, P).transpose(1, 2, 0).reshape(SHP, OUT)
        outs.append(full_m[node_pos[m * SH:(m + 1) * SH]])
    full = np.concatenate(outs, axis=0).astype(np.float32)
    if _return_results:
        return full, res
    return full


# revision 17
# speedup vs baseline: 1.1702x; 1.1702x over previous
"""GCN message-passing layer (GCNConv + skip + BatchNorm + ReLU) on 8 TRN2 cores.

Strategy v3 (CSR segment-sum, transposed output, no on-device gather):
  - Nodes sharded across 8 cores (12500 each, padded to 12544 = 98*128),
    placed degree-descending into 128-row tiles so each tile's max
    in-degree K_t is near its mean (CSR padding ~10%).
  - Host prep (integer indexing / byte layout only): edges+self-loops CSR
    grouped by target; per TILE-PAIR the source rows of x (bf16) are laid
    out dense [128 slots, 2, 64 feats, Kp] (j contiguous), shipped with
    per-edge source degrees (1e30 padding => rsqrt ~ 0 kills pads).
  - Device per pair: one DMA; one tensor_tensor mult by dinv_src
    (broadcast over feats; alternates GPSIMD/DVE); ONE 2x-mode bf16
    tensor_reduce over K = the segment sum for both tiles. Per tile: PE
    matmul agg^T @ diag(dinv_tgt) (transpose + target normalization in
    one), evac into stacked [aggT ; xT]; PE matmul lhsT=[W;skipW] gives
    v^T = (agg@W + x@skipW)^T with BN feature dim on partitions; ACT evac
    with free accum_out = BN sum. Sum-of-squares, BN AllReduce, and the
    affine+ReLU apply run as a handful of wide whole-buffer ops.
  - All float arithmetic runs on device; the host only reorders input
    bytes (bf16 cast) and computes integer degrees (+ 1/sqrt(deg) diag,
    same class as the baseline's host-built float index tables).
"""

import numpy as np
import ml_dtypes

P = 128
_BF16 = ml_dtypes.bfloat16

_KCACHE = {}


def _host_prep(x, edge_index, W, skip_W, gamma, beta, M, IN, OUT):
    N = x.shape[0]
    SH = N // M
    T = -(-SH // P)
    SHP = T * P
    assert T % 2 == 0
    NP = T // 2

    row = edge_index[0].astype(np.int64)
    col = edge_index[1].astype(np.int64)
    loops = np.arange(N, dtype=np.int64)
    row_f = np.concatenate([row, loops])
    col_f = np.concatenate([col, loops])

    deg_i = np.bincount(col_f, minlength=N)          # >=1 (self loops)
    deg_f = deg_i.astype(np.float32)

    # degree-descending node placement per core: rank r -> (tile r//P, slot r%P)
    node_pos = np.empty(N, dtype=np.int64)
    orders = []
    Kt_cores = []
    for m in range(M):
        dg = deg_i[m * SH:(m + 1) * SH]
        order = np.argsort(-dg, kind="stable")
        ranks = np.empty(SH, dtype=np.int64)
        ranks[order] = np.arange(SH)
        node_pos[m * SH:(m + 1) * SH] = ranks
        orders.append(order)
        dgs = np.zeros(SHP, dtype=np.int64)
        dgs[:SH] = dg[order]
        Kt_cores.append(dgs.reshape(T, P).max(axis=1))
    Kt = np.maximum.reduce(Kt_cores)
    Kp = np.maximum(Kt.reshape(NP, 2).max(axis=1), 4)
    Kp = ((Kp + 3) // 4) * 4                         # per-PAIR K, mult of 4
    opf = np.zeros(NP + 1, dtype=np.int64)
    np.cumsum(Kp, out=opf[1:])
    SKP = int(opf[-1])                               # sum of pair Ks

    # CSR by target node
    eorder = np.argsort(col_f, kind="stable")
    row_s = row_f[eorder]
    starts = np.zeros(N + 1, dtype=np.int64)
    np.cumsum(deg_i, out=starts[1:])

    x_bf = x.astype(_BF16)
    WS = np.concatenate([np.asarray(W), np.asarray(skip_W)], axis=0).astype(_BF16)
    dinv_all = (1.0 / np.sqrt(deg_f)).astype(np.float32)

    in_maps = []
    for m in range(M):
        order = orders[m]
        xgt = np.zeros((P, 2 * IN * SKP), dtype=_BF16)
        dege = np.full((P, 2 * SKP), 1e30, dtype=np.float32)
        diag = np.zeros((P, T * P), dtype=_BF16)
        ii = np.arange(P)
        for t in range(T):
            K = int(Kp[t // 2])
            o2 = int(2 * IN * opf[t // 2] + (t % 2) * IN * K)
            od = int(2 * opf[t // 2] + (t % 2) * K)
            rr = np.arange(t * P, (t + 1) * P)
            vslot = rr < SH
            ln = np.where(vslot, order[np.minimum(rr, SH - 1)], 0)
            gn = m * SH + ln
            cnt = np.where(vslot, deg_i[gn], 0)
            st = starts[gn]
            j = np.arange(K, dtype=np.int64)[None, :]
            vm = j < cnt[:, None]                    # [P, K]
            eidx = st[:, None] + np.minimum(j, np.maximum(cnt[:, None] - 1, 0))
            srcs = np.where(vm, row_s[eidx], 0)
            xg = x_bf[srcs]                          # [P, K, IN]
            xgt[:, o2:o2 + IN * K] = xg.transpose(0, 2, 1).reshape(P, IN * K)
            dege[:, od:od + K] = np.where(vm, deg_f[srcs], 1e30)
            dv_t = np.where(vslot, dinv_all[gn], 1.0).astype(_BF16)
            diag[ii, t * P + ii] = dv_t

        # stacked transposed x: partitions IN..2*IN hold x^T at permuted slots
        xperm = np.zeros((SHP, IN), dtype=_BF16)
        xperm[node_pos[m * SH:(m + 1) * SH]] = x_bf[m * SH:(m + 1) * SH]
        xstack = np.zeros((P, T * P), dtype=_BF16)
        xstack[IN:2 * IN, :] = xperm.T

        in_maps.append({
            "xgt": np.ascontiguousarray(xgt),
            "dege": np.ascontiguousarray(dege),
            "xstack": np.ascontiguousarray(xstack),
            "diag": np.ascontiguousarray(diag),
            "WS": np.ascontiguousarray(WS),
            "gammac": np.ascontiguousarray(np.asarray(gamma, np.float32).reshape(1, OUT)),
            "betac": np.ascontiguousarray(np.asarray(beta, np.float32).reshape(1, OUT)),
        })
    return in_maps, tuple(int(k) for k in Kp), node_pos, SH, T, SHP


def _build(M, N, IN, OUT, T, Kp, debug_stop="full"):
    from concourse import bacc, mybir, tile

    dt = mybir.dt
    Alu = mybir.AluOpType
    Act = mybir.ActivationFunctionType

    SHP = T * P
    NP = T // 2
    BN_EPS = 1e-5
    opf = np.zeros(NP + 1, dtype=np.int64)
    np.cumsum(np.asarray(Kp), out=opf[1:])
    SKP = int(opf[-1])

    nc = bacc.Bacc("TRN2", target_bir_lowering=False, debug=False,
                   num_devices=M)

    xgt_d = nc.dram_tensor("xgt", [P, 2 * IN * SKP], dt.bfloat16,
                           kind="ExternalInput")
    dege_d = nc.dram_tensor("dege", [P, 2 * SKP], dt.float32,
                            kind="ExternalInput")
    xstack_d = nc.dram_tensor("xstack", [P, T * P], dt.bfloat16,
                              kind="ExternalInput")
    diag_d = nc.dram_tensor("diag", [P, T * P], dt.bfloat16,
                            kind="ExternalInput")
    WS_d = nc.dram_tensor("WS", [2 * IN, OUT], dt.bfloat16, kind="ExternalInput")
    gammac_d = nc.dram_tensor("gammac", [1, OUT], dt.float32,
                              kind="ExternalInput")
    betac_d = nc.dram_tensor("betac", [1, OUT], dt.float32,
                             kind="ExternalInput")
    out_d = nc.dram_tensor("out", [P, T * P], dt.bfloat16,
                           kind="ExternalOutput")

    st_local = nc.dram_tensor("st_local", [2, OUT], dt.float32)
    st_global = nc.dram_tensor("st_global", [2, OUT], dt.float32,
                               addr_space="Shared")
    sta_local = nc.dram_tensor("sta_local", [2, OUT], dt.float32)
    sta_global = nc.dram_tensor("sta_global", [2, OUT], dt.float32,
                                addr_space="Shared")
    rg = [list(range(M))]

    NCH = 7                       # sum-of-squares chunking
    assert T % NCH == 0
    CH = T // NCH

    with tile.TileContext(nc) as tc:
        with (
            tc.tile_pool(name="const", bufs=1) as cpool,
            tc.tile_pool(name="gload", bufs=3) as gpool,
            tc.tile_pool(name="gmul", bufs=2) as mpool,
            tc.tile_pool(name="hfold", bufs=2) as hpool,
            tc.tile_pool(name="aggp", bufs=3) as apool,
            tc.tile_pool(name="sqp", bufs=2) as qpool,
            tc.tile_pool(name="ps_tr", bufs=2, space="PSUM") as ps_tr,
            tc.tile_pool(name="ps_out", bufs=3, space="PSUM") as ps_out,
        ):
            # ---- constants / persistent ----
            xstack_sb = cpool.tile([P, T * P], dt.bfloat16, tag="xstack")
            diag_sb = cpool.tile([P, T * P], dt.bfloat16, tag="diag")
            cq = T * P // 4
            for qi in range(4):
                nc.sync.dma_start(diag_sb[:, qi * cq:(qi + 1) * cq],
                                  diag_d[:, qi * cq:(qi + 1) * cq])
                nc.sync.dma_start(xstack_sb[:, qi * cq:(qi + 1) * cq],
                                  xstack_d[:, qi * cq:(qi + 1) * cq])
            WS_sb = cpool.tile([2 * IN, OUT], dt.bfloat16, tag="WS")
            nc.sync.dma_start(WS_sb[:], WS_d[:, :])
            dege_sb = cpool.tile([P, 2 * SKP], dt.float32, tag="dege")
            nc.sync.dma_start(dege_sb[:], dege_d[:, :])
            gammar_sb = cpool.tile([1, OUT], dt.float32, tag="gammar")
            nc.sync.dma_start(gammar_sb[:], gammac_d[:, :])
            betar_sb = cpool.tile([1, OUT], dt.float32, tag="betar")
            nc.sync.dma_start(betar_sb[:], betac_d[:, :])
            from concourse.masks import make_identity
            identf = cpool.tile([P, P], dt.float32, tag="identf")
            make_identity(nc, identf[:])
            ones11 = cpool.tile([1, 1], dt.float32, tag="ones11")
            nc.vector.memset(ones11[:], 1.0)

            vbuf = cpool.tile([P, T * P], dt.bfloat16, tag="vbuf")
            st2a = cpool.tile([P, 2], dt.float32, tag="st2a")
            accs = cpool.tile([P, T], dt.float32, tag="accs")
            accq = cpool.tile([P, NCH], dt.float32, tag="accq")

            # dinv_src = sqrt(1/deg) (ACT Rsqrt banned for accuracy)
            rec_e = cpool.tile([P, 2 * SKP], dt.float32, tag="rec_e")
            dinve = cpool.tile([P, 2 * SKP], dt.bfloat16, tag="dinve")
            dq = 2 * SKP // 4
            for qi in range(4):
                lo = qi * dq
                hi = 2 * SKP if qi == 3 else (qi + 1) * dq
                nc.vector.reciprocal(rec_e[:, lo:hi], dege_sb[:, lo:hi])
                nc.scalar.activation(dinve[:, lo:hi], rec_e[:, lo:hi],
                                     Act.Sqrt)

            # ---- main loop over tile pairs ----
            def stage_front(p):
                K = int(Kp[p])
                o = int(opf[p])
                g = gpool.tile([P, 2, IN, K], dt.bfloat16, tag="g")
                nc.sync.dma_start(g[:], xgt_d[:, 2 * IN * o:2 * IN * (o + K)])
                gm = mpool.tile([P, 2, IN, K], dt.bfloat16, tag="gm")
                dv = (dinve[:, 2 * o:2 * (o + K)]
                      .rearrange("p (two k) -> p two k", two=2)
                      .unsqueeze(2).broadcast_to([P, 2, IN, K]))
                nc.vector.tensor_tensor(gm[:], g[:], dv, Alu.mult)
                hq[p] = gm

            hq = [None] * NP

            def stage_back(p):
                gm = hq[p]
                hq[p] = None
                aggp = apool.tile([P, 2 * IN], dt.bfloat16, tag="aggp")
                with nc.allow_low_precision("bf16 agg; 2x-mode reduce"):
                    nc.vector.tensor_reduce(aggp[:], gm[:],
                                            mybir.AxisListType.X, Alu.add)
                for half in (0, 1):
                    ti = 2 * p + half
                    pt = ps_tr.tile([IN, P], dt.float32, tag="pt")
                    nc.tensor.matmul(pt[:],
                                     lhsT=aggp[:, half * IN:(half + 1) * IN],
                                     rhs=diag_sb[:, ti * P:(ti + 1) * P],
                                     start=True, stop=True)
                    nc.scalar.copy(xstack_sb[0:IN, ti * P:(ti + 1) * P], pt[:])
                    po = ps_out.tile([P, P], dt.float32, tag="po")
                    nc.tensor.matmul(po[:], lhsT=WS_sb[:],
                                     rhs=xstack_sb[:, ti * P:(ti + 1) * P],
                                     start=True, stop=True)
                    nc.scalar.activation(vbuf[:, ti * P:(ti + 1) * P], po[:],
                                         Act.Copy,
                                         accum_out=accs[:, ti:ti + 1])
                if debug_stop == "full" and (2 * p + 2) % CH == 0:
                    c = (2 * p + 2) // CH - 1
                    scr = qpool.tile([P, CH * P], dt.bfloat16, tag="scr")
                    vsl = vbuf[:, c * CH * P:(c + 1) * CH * P]
                    nc.scalar.activation(scr[:], vsl, Act.Square,
                                         accum_out=accq[:, c:c + 1])

            TSPLIT = 84
            for p in range(NP):
                stage_front(p)
                stage_back(p)
                if debug_stop == "full" and 2 * p + 2 == TSPLIT:
                    # early partial-stats collective (also absorbs core skew)
                    nc.vector.tensor_reduce(st2a[:, 0:1], accs[:, 0:TSPLIT],
                                            mybir.AxisListType.X, Alu.add)
                    nc.vector.tensor_reduce(
                        st2a[:, 1:2], accq[:, 0:TSPLIT // CH],
                        mybir.AxisListType.X, Alu.add)
                    psta = ps_tr.tile([2, P], dt.float32, tag="pst", bufs=1)
                    nc.tensor.matmul(psta[:], lhsT=st2a[:], rhs=identf[:],
                                     start=True, stop=True)
                    sta_sb = cpool.tile([2, P], dt.float32, tag="sta_sb")
                    nc.scalar.copy(sta_sb[:], psta[:])
                    nc.sync.dma_start(sta_local[:, :], sta_sb[:])
                    nc.gpsimd.collective_compute(
                        "AllReduce", Alu.add, replica_groups=rg,
                        ins=[sta_local.ap().opt()],
                        outs=[sta_global.ap().opt()])

            if debug_stop == "v":
                nc.sync.dma_start(out_d[:, :], vbuf[:])

            if debug_stop == "full":
                # ---- BN stats: sum of squares (chunked), totals, allreduce
                st2 = cpool.tile([P, 2], dt.float32, tag="st2")
                nc.vector.tensor_reduce(st2[:, 0:1], accs[:, TSPLIT:T],
                                        mybir.AxisListType.X, Alu.add)
                nc.vector.tensor_reduce(st2[:, 1:2],
                                        accq[:, TSPLIT // CH:NCH],
                                        mybir.AxisListType.X, Alu.add)
                # -> row layout [2, P] so the collective sees a flat buffer
                pst = ps_tr.tile([2, P], dt.float32, tag="pst", bufs=1)
                nc.tensor.matmul(pst[:], lhsT=st2[:], rhs=identf[:],
                                 start=True, stop=True)
                st_sb = cpool.tile([2, P], dt.float32, tag="st_sb")
                nc.scalar.copy(st_sb[:], pst[:])
                nc.sync.dma_start(st_local[:, :], st_sb[:])
                nc.gpsimd.collective_compute(
                    "AllReduce", Alu.add, replica_groups=rg,
                    ins=[st_local.ap().opt()], outs=[st_global.ap().opt()])
                sg_sum = cpool.tile([1, OUT], dt.float32, tag="sg_sum")
                nc.sync.dma_start(sg_sum[:], st_global[0:1, :])
                sg_sq = cpool.tile([1, OUT], dt.float32, tag="sg_sq")
                nc.sync.dma_start(sg_sq[:], st_global[1:2, :])
                sga_sum = cpool.tile([1, OUT], dt.float32, tag="sga_sum")
                nc.sync.dma_start(sga_sum[:], sta_global[0:1, :])
                sga_sq = cpool.tile([1, OUT], dt.float32, tag="sga_sq")
                nc.sync.dma_start(sga_sq[:], sta_global[1:2, :])
                nc.vector.tensor_tensor(sg_sum[:], sg_sum[:], sga_sum[:],
                                        Alu.add)
                nc.vector.tensor_tensor(sg_sq[:], sg_sq[:], sga_sq[:],
                                        Alu.add)

                # BN coefficient math in row form [1, OUT]
                inv_n = 1.0 / float(N)
                mean_r = cpool.tile([1, OUT], dt.float32, tag="mean_r")
                nc.vector.tensor_scalar(mean_r[:], sg_sum[:], inv_n,
                                        None, Alu.mult)
                var_r = cpool.tile([1, OUT], dt.float32, tag="var_r")
                nc.vector.tensor_scalar(var_r[:], sg_sq[:], inv_n,
                                        None, Alu.mult)
                msq = cpool.tile([1, OUT], dt.float32, tag="msq")
                nc.vector.tensor_tensor(msq[:], mean_r[:], mean_r[:], Alu.mult)
                nc.vector.tensor_tensor(var_r[:], var_r[:], msq[:],
                                        Alu.subtract)
                nc.vector.tensor_scalar(var_r[:], var_r[:], BN_EPS, None,
                                        Alu.add)
                rvar = cpool.tile([1, OUT], dt.float32, tag="rvar")
                nc.vector.reciprocal(rvar[:], var_r[:])
                rstd = cpool.tile([1, OUT], dt.float32, tag="rstd")
                nc.scalar.activation(rstd[:], rvar[:], Act.Sqrt)
                a_row = cpool.tile([1, OUT], dt.float32, tag="a_row")
                nc.vector.tensor_tensor(a_row[:], gammar_sb[:], rstd[:],
                                        Alu.mult)
                ma = cpool.tile([1, OUT], dt.float32, tag="ma")
                nc.vector.tensor_tensor(ma[:], mean_r[:], a_row[:], Alu.mult)
                b_row = cpool.tile([1, OUT], dt.float32, tag="b_row")
                nc.vector.tensor_tensor(b_row[:], betar_sb[:], ma[:],
                                        Alu.subtract)
                # rows -> per-partition columns via 1-contraction matmuls
                pa = ps_out.tile([P, 1], dt.float32, tag="pa", bufs=1)
                nc.tensor.matmul(pa[:], lhsT=a_row[:], rhs=ones11[:],
                                 start=True, stop=True)
                a_col = cpool.tile([P, 1], dt.float32, tag="a_col")
                nc.scalar.copy(a_col[:], pa[:])
                pb = ps_tr.tile([P, 1], dt.float32, tag="pb", bufs=1)
                nc.tensor.matmul(pb[:], lhsT=b_row[:], rhs=ones11[:],
                                 start=True, stop=True)
                b_col = cpool.tile([P, 1], dt.float32, tag="b_col")
                nc.scalar.copy(b_col[:], pb[:])

                # ---- BN apply + ReLU in-place, batched output DMA ----
                nc.vector.tensor_scalar(vbuf[:], vbuf[:], a_col[:], b_col[:],
                                        Alu.mult, Alu.add)
                nc.vector.tensor_scalar(vbuf[:], vbuf[:], 0.0, None, Alu.max)
                H = T * P // 2
                nc.sync.dma_start(out_d[:, 0:H], vbuf[:, 0:H])
                nc.sync.dma_start(out_d[:, H:2 * H], vbuf[:, H:2 * H])

    nc.compile()
    return nc


def _run(nc, in_maps, M, trace=False):
    from concourse import bass_utils
    res = bass_utils.run_bass_kernel_spmd(
        nc, in_maps, core_ids=list(range(M)), trace=trace)
    return res


def kernel(x, edge_index, W, bias, skip_W, gamma, beta, _trace=False,
           _return_results=False, _debug_stop="full"):
    x = np.asarray(x, dtype=np.float32)
    edge_index = np.asarray(edge_index, dtype=np.int32)
    M = 8
    N, IN = x.shape
    OUT = np.asarray(W).shape[1]

    in_maps, Kp, node_pos, SH, T, SHP = _host_prep(
        x, edge_index, W, skip_W, gamma, beta, M, IN, OUT)
    key = (M, N, IN, OUT, T, Kp, _debug_stop)
    if key not in _KCACHE:
        _KCACHE[key] = _build(M, N, IN, OUT, T, Kp, debug_stop=_debug_stop)
    nc = _KCACHE[key]

    res = _run(nc, in_maps, M, trace=_trace)
    outs = []
    for m in range(M):
        arr = res.results[m]["out"].astype(np.float32)      # [P, T*P] v^T
        full_m = arr.reshape(OUT, T, P).transpose(1, 2, 0).reshape(SHP, OUT)
        outs.append(full_m[node_pos[m * SH:(m + 1) * SH]])
    full = np.concatenate(outs, axis=0).astype(np.float32)
    if _return_results:
        return full, res
    return full
